# revision 1
# baseline (speedup 1.0000x reference)
#!/usr/bin/env python3
"""Multi-head attention (B=16, N=1024, E=768, H=8, softmax-then-scale variant)
as a Bass/Tile kernel on 8 TRN2 NeuronCores, data-parallel over the batch.

Per core (2 batch elements, T=2048 tokens), all matmuls in fp32r (full-rate
PE with ~2^-15 mantissa rounding; measured matmul relerr 3e-5 vs fp32):
  - x fed pre-transposed from host as xT [E, T]; activation/weight DRAM
    tensors are declared float32r so DMA loads them directly (the PE
    truncates the mantissa on read - verified equivalent on HW).
  - loop over batch b, then head h:
      Q^T/K^T: lhsT = Wq[:,h] slice [128,96], rhs = xT chunk -> [96, 1024]
      energy^T per ktile: lhsT = K^T slice [96,128], rhs = Q^T [96,512]
      exp on ScalarE (no max subtraction: |energy| <~ 60 fits fp32 exp)
      attn@V flash-style: lhsT = Vhat [128, 97] (V cols for head h + a
        sqrt(E) constant column so row 96 accumulates sqrt(E)*sumexp),
        rhs = expT [128,512], accumulated over 8 k-tiles -> zT [97, 1024]
      normalize: recip = 1/zT[96] (DVE), replicated across partitions by
        the gpsimd partition_broadcast custom op, z_h = zT[0:96] * recip
    then output projection for batch b: R = sum_h z_h^T.T @ Wo_h + 1^T bo
"""
import os
import sys

sys.path.insert(0, "/opt/trn_rl_repo")

import numpy as np

B, N, E, H, D = 16, 1024, 768, 8, 96
NCORES = 8
BPC = B // NCORES          # batch elements per core
T = BPC * N                # tokens per core
KT = E // 128              # k-tiles over embedding dim (6)
MT = T // 128              # token tiles per core (16)
NKT = N // 128             # k-tiles over sequence (8)

_CACHE = {}


def _build(with_bias=True):
    import concourse.tile as tile
    from concourse import bacc, mybir

    f32 = mybir.dt.float32
    f32r = mybir.dt.float32r

    nc = bacc.Bacc("TRN2", target_bir_lowering=False, debug=False)

    # activation/weight inputs are declared float32r: the PE truncates the
    # mantissa on read, so feeding raw fp32 bits through DMA is equivalent
    # to an on-chip rounding pass (verified on HW)
    xT_d = nc.dram_tensor("xT", [E, T], f32r, kind="ExternalInput").ap()
    wq_d = nc.dram_tensor("wqh", [H, 128, KT, D], f32r, kind="ExternalInput").ap()
    wk_d = nc.dram_tensor("wkh", [H, 128, KT, D], f32r, kind="ExternalInput").ap()
    wv_d = nc.dram_tensor("wv", [E, E], f32r, kind="ExternalInput").ap()
    wo_d = nc.dram_tensor("wo", [E, E], f32r, kind="ExternalInput").ap()
    bqk_d = nc.dram_tensor("bqk", [D, 2 * H], f32, kind="ExternalInput").ap()
    bv_d = nc.dram_tensor("bv1", [1, E], f32r, kind="ExternalInput").ap()
    bo_d = nc.dram_tensor("bo1", [1, E], f32r, kind="ExternalInput").ap()
    out_d = nc.dram_tensor("out", [T, E], f32, kind="ExternalOutput").ap()

    with tile.TileContext(nc) as tc:
        _body(nc, tc, mybir,
              xT_d, wq_d, wk_d, wv_d, wo_d, bqk_d, bv_d, bo_d, out_d,
              with_bias)

    nc.compile()
    return nc


def _body(nc, tc, mybir,
          xT_d, wq_d, wk_d, wv_d, wo_d, bqk_d, bv_d, bo_d, out_d,
          with_bias):
    from contextlib import ExitStack
    from concourse import library_config
    from concourse.tile import add_dep_helper

    f32 = mybir.dt.float32
    f32r = mybir.dt.float32r
    Exp = mybir.ActivationFunctionType.Exp
    ADD = mybir.AluOpType.add
    SQRT_E = float(np.float32(np.sqrt(E)))

    ctx = ExitStack()
    with ctx:
        persist = ctx.enter_context(tc.tile_pool(name="persist", bufs=1))
        qkpool = ctx.enter_context(tc.tile_pool(name="qkpool", bufs=1))
        wqkpool = ctx.enter_context(tc.tile_pool(name="wqkpool", bufs=1))
        projp = ctx.enter_context(tc.tile_pool(name="projp", bufs=2, space="PSUM"))
        dramp = ctx.enter_context(tc.tile_pool(name="dramp", bufs=2, space="DRAM"))
        epp = ctx.enter_context(tc.tile_pool(name="epp", bufs=2, space="PSUM"))
        zp = ctx.enter_context(tc.tile_pool(name="zp", bufs=2, space="PSUM"))

        xt = []
        vhat = []
        wo8 = []
        state = {}

        # ---------------- helpers ----------------
        def proj_head(b, h):
            """Load Wq/Wk slices for head h, compute Q^T/K^T for batch b."""
            tok0 = b * N
            wqr = {}
            for nm, wd in (("q", wq_d), ("k", wk_d)):
                wr = wqkpool.tile([128, KT, D], f32r, name=f"w{nm}r",
                                  tag=f"w{nm}r", bufs=2)
                nc.gpsimd.dma_start(out=wr, in_=wd[h])
                wqr[nm] = wr

            qk = {}
            for i, nm in enumerate(("q", "k")):
                qt = qkpool.tile([D, N], f32r, name=f"{nm}t", tag=f"{nm}t",
                                 bufs=2)
                for tc2 in range(N // 512):
                    pq = projp.tile([128, 512], f32, name="pp", tag="pp")
                    for c in range(KT):
                        nc.tensor.matmul(
                            pq[0:D, :],
                            wqr[nm][:, c, :],
                            xt[c][:, tok0 + tc2 * 512:tok0 + (tc2 + 1) * 512],
                            start=(c == 0), stop=(c == KT - 1),
                        )
                    if with_bias:
                        cp = nc.vector.tensor_scalar(
                            out=qt[:, tc2 * 512:(tc2 + 1) * 512],
                            in0=pq[0:D, :],
                            scalar1=state["bqk_t"][:, i * H + h:i * H + h + 1],
                            scalar2=None, op0=ADD,
                        )
                    else:
                        cp = nc.vector.tensor_copy(
                            out=qt[:, tc2 * 512:(tc2 + 1) * 512],
                            in_=pq[0:D, :],
                        )
                    qk["last_cp"] = cp
                qk[nm] = qt
            return qk

        def attention(b, h, qk):
            """energy -> exp -> attn@V -> normalized z for (b, h)."""
            zT = zp.tile([128, N], f32, name="zT", tag="zT")
            for kt in range(NKT):
                ext = expp.tile([128, N], f32r, name="ext", tag="ext")
                for qc in range(2):
                    ep = epp.tile([128, 512], f32, name="ep", tag="ep")
                    nc.tensor.matmul(
                        ep,
                        qk["k"][:, kt * 128:(kt + 1) * 128],
                        qk["q"][:, qc * 512:(qc + 1) * 512],
                        start=True, stop=True,
                    )
                    nc.scalar.activation(
                        out=ext[:, qc * 512:(qc + 1) * 512], in_=ep, func=Exp)
                    nc.tensor.matmul(
                        zT[0:D + 1, qc * 512:(qc + 1) * 512],
                        vhat[b * NKT + kt][:, h, :],
                        ext[:, qc * 512:(qc + 1) * 512],
                        start=(kt == 0), stop=(kt == NKT - 1),
                    )

            # normalize: z = zT[0:D] / zT[D]  (row D = sqrt(E)*sumexp),
            # split per 512-column half; the recip row is replicated across
            # partitions with the gpsimd partition_broadcast custom
            # instruction (SBUF->SBUF, no DRAM round-trip)
            zth = ztpool.tile([D, N], f32r, name=f"zt{h}", tag=f"zt{h}")
            for qc in range(2):
                sl = slice(qc * 512, (qc + 1) * 512)
                recip = rbp.tile([1, 512], f32, name="recip", tag="recip",
                                 bufs=2)
                nc.vector.reciprocal(out=recip, in_=zT[D:D + 1, sl])
                rb = rbp.tile([D, 512], f32, name="rb", tag="rb")
                nc.gpsimd.partition_broadcast(out_ap=rb, in_ap=recip)
                nc.vector.tensor_mul(out=zth[:, sl], in0=zT[0:D, sl], in1=rb)
            return zth

        def final_proj(b, zt8):
            """Output projection, software-pipelined across 5 psum groups.

            Heads 0..6 of up to 5 (mt, half) groups are accumulated before
            the first h7 matmul, so the PE has ~5us of work while the last
            head's normalize chain (recip -> DRAM round-trip -> mul) is
            still producing zt8[7]. Slots are borrowed from the idle
            energy (ep) and attention-accumulator (zT) pools.
            """
            tok0 = b * N
            groups = [(mt, half) for mt in range(NKT) for half in range(2)]
            DEPTH = 5
            prs = {}
            ros = {}

            def open_group(g):
                mt, half = groups[g]
                k = g % DEPTH
                if k < 2:
                    pr = projp.tile([128, 384], f32, name="pp", tag="pp")
                elif k < 4:
                    pr = epp.tile([128, 384], f32, name="fep", tag="ep")
                else:
                    pr = zp.tile([128, 384], f32, name="fzt", tag="zT")
                cols = slice(half * 384, (half + 1) * 384)
                for h in range(H - 1):
                    nc.tensor.matmul(
                        pr, zt8[h][:, mt * 128:(mt + 1) * 128], wo8[h][:, cols],
                        start=(h == 0), stop=False,
                    )
                prs[g] = pr

            for g in range(min(DEPTH, len(groups))):
                open_group(g)
            for g, (mt, half) in enumerate(groups):
                pr = prs.pop(g)
                cols = slice(half * 384, (half + 1) * 384)
                nc.tensor.matmul(
                    pr, zt8[H - 1][:, mt * 128:(mt + 1) * 128],
                    wo8[H - 1][:, cols],
                    start=False, stop=(not with_bias),
                )
                if with_bias:
                    nc.tensor.matmul(
                        pr, onescol_r, state["bor"][:, cols],
                        start=False, stop=True,
                    )
                if half == 0:
                    ros[mt] = rop.tile([128, E], f32, name="ro", tag="ro")
                if g % 2 == 0:
                    nc.scalar.copy(out=ros[mt][:, cols], in_=pr)
                else:
                    nc.vector.tensor_copy(out=ros[mt][:, cols], in_=pr)
                if g + DEPTH < len(groups):
                    open_group(g + DEPTH)
                # ship each half as soon as its copy lands
                nc.sync.dma_start(
                    out=out_d[tok0 + mt * 128:tok0 + (mt + 1) * 128, cols],
                    in_=ros[mt][:, cols])
                if half == 1:
                    ros.pop(mt)

        # ---------------- phase 0: loads + Vhat ----------------
        qk00 = None
        with tc.tile_pool(name="wvpool", bufs=1) as wvpool:
            for c in range(KT):
                xtc = persist.tile([128, T], f32r, name=f"xt{c}", tag=f"xt{c}")
                xt.append(xtc)

            def load_x_quarter(q):
                for hf in range(2):
                    sl = slice(q * 512 + hf * 256, q * 512 + (hf + 1) * 256)
                    for c in range(KT):
                        nc.sync.dma_start(
                            out=xt[c][:, sl],
                            in_=xT_d[c * 128:(c + 1) * 128, sl])

            # constants
            ones_f = persist.tile([1, 128], f32, name="ones_f", tag="ones_f")
            nc.vector.memset(ones_f, 1.0)
            onescol_r = persist.tile([1, 128], f32r, name="ones_r", tag="ones_r")
            nc.vector.tensor_copy(out=onescol_r, in_=ones_f)
            c27f = persist.tile([128, 1], f32, name="c27f", tag="c27f")
            nc.vector.memset(c27f, SQRT_E)
            c27r = persist.tile([128, 1], f32r, name="c27r", tag="c27r")
            nc.vector.tensor_copy(out=c27r, in_=c27f)

            # first x quarter interleaved with Wv so the Vhat(0) psum
            # group can start accumulating after the first (x, wv) pair;
            # loaded in 256-column halves so Vhat(mt0/mt1) unblock early
            wv = []
            for c in range(KT):
                nc.sync.dma_start(
                    out=xt[c][:, 0:256], in_=xT_d[c * 128:(c + 1) * 128, 0:256])
                wvc = wvpool.tile([128, E], f32r, name=f"wv{c}", tag=f"wv{c}")
                nc.gpsimd.dma_start(out=wvc, in_=wv_d[c * 128:(c + 1) * 128, :])
                wv.append(wvc)
            for c in range(KT):
                nc.sync.dma_start(
                    out=xt[c][:, 256:512],
                    in_=xT_d[c * 128:(c + 1) * 128, 256:512])

            # gpsimd ucode library with partition_broadcast (needed by the
            # first normalize ~35us in; emitted after the Wv loads so it
            # does not head-of-line block the gpsimd DMA queue at startup)
            nc.gpsimd.load_library(library_config.attn)

            # biases
            bqk_t = persist.tile([D, 2 * H], f32, name="bqk_t", tag="bqk_t")
            nc.gpsimd.dma_start(out=bqk_t, in_=bqk_d)
            state["bqk_t"] = bqk_t
            bvr = persist.tile([1, E], f32r, name="bvr", tag="bvr")
            nc.gpsimd.dma_start(out=bvr, in_=bv_d)

            def build_vhat(mt):
                # Vhat[mt] : [128 tokens, H, D+1]; column D holds sqrt(E)
                vh = persist.tile([128, H, D + 1], f32r, name=f"vhat{mt}",
                                  tag=f"vhat{mt}")
                for half in range(2):  # heads 0-3 / 4-7 (384 cols each)
                    pv = projp.tile([128, 512], f32, name="pp", tag="pp")
                    cols = slice(half * 4 * D, (half + 1) * 4 * D)
                    for c in range(KT):
                        nc.tensor.matmul(
                            pv[:, 0:4 * D],
                            xt[c][:, mt * 128:(mt + 1) * 128],
                            wv[c][:, cols],
                            start=(c == 0),
                            stop=(not with_bias and c == KT - 1),
                        )
                    if with_bias:
                        nc.tensor.matmul(
                            pv[:, 0:4 * D], onescol_r, bvr[:, cols],
                            start=False, stop=True,
                        )
                    nc.scalar.copy(
                        out=vh[:, half * 4:(half + 1) * 4, 0:D],
                        in_=pv[:, 0:4 * D].rearrange("p (h d) -> p h d", h=4),
                    )
                nc.vector.tensor_copy(
                    out=vh[:, :, D:D + 1],
                    in_=c27r.to_broadcast([128, H, 1]),
                )
                vhat.append(vh)

            # interleave: quarters 0-1 -> Vhat 0-7, then the first head
            # projection (keeps the PE busy while quarters 2-3 stream in)
            for q in range(2):
                if q > 0:
                    load_x_quarter(q)
                for mt in range(4 * q, 4 * q + 4):
                    build_vhat(mt)
            qk00 = proj_head(0, 0)
            for q in range(2, 4):
                load_x_quarter(q)
                for mt in range(4 * q, 4 * q + 4):
                    build_vhat(mt)

        # stage + wv pools released; later pools reuse their space
        expp = ctx.enter_context(tc.tile_pool(name="expp", bufs=3))
        rbp = ctx.enter_context(tc.tile_pool(name="rbp", bufs=2))
        rop = ctx.enter_context(tc.tile_pool(name="rop", bufs=2))
        ztpool = ctx.enter_context(tc.tile_pool(name="ztpool", bufs=1))
        wopool = ctx.enter_context(tc.tile_pool(name="wopool", bufs=1))

        # Wo -> fp32r per-head tiles + bo (phase 2 operands)
        for h in range(H):
            woh = wopool.tile([D, E], f32r, name=f"wo{h}", tag=f"wo{h}")
            nc.gpsimd.dma_start(out=woh, in_=wo_d[h * D:(h + 1) * D, :])
            wo8.append(woh)
        if with_bias:
            bor = wopool.tile([1, E], f32r, name="bor", tag="bor")
            nc.gpsimd.dma_start(out=bor, in_=bo_d)
            state["bor"] = bor

        # ---------------- phases 1+2, batch-major, software-pipelined ------
        qk_next = qk00
        for b in range(BPC):
            zt8 = []
            for h in range(H):
                qk = qk_next if (h == 0 and qk_next is not None) \
                    else proj_head(b, h)
                qk_next = None
                zt8.append(attention(b, h, qk))
            if b + 1 < BPC:
                # emit next batch's first projection before the output
                # projection so the PE has work while zt(h=7) normalizes
                qk_next = proj_head(b + 1, 0)
            final_proj(b, zt8)


def _get_runner(with_bias=False):
    """Build (once per variant) a jitted shard_map executing the NEFF."""
    key = ("runner", with_bias)
    if key in _CACHE:
        return _CACHE[key]

    import jax
    from jax.experimental.shard_map import shard_map
    from jax.sharding import Mesh, NamedSharding, PartitionSpec
    from concourse import mybir
    from concourse.bass2jax import (
        _bass_exec_p, install_neuronx_cc_hook, partition_id_tensor)

    nc = _build(with_bias=with_bias)
    install_neuronx_cc_hook()

    partition_name = (
        nc.partition_id_tensor.name if nc.partition_id_tensor else None)
    in_names, out_names, out_avals, zero_outs = [], [], [], []
    for alloc in nc.m.functions[0].allocations:
        if not isinstance(alloc, mybir.MemoryLocationSet):
            continue
        name = alloc.memorylocations[0].name
        if alloc.kind == "ExternalInput":
            if name != partition_name:
                in_names.append(name)
        elif alloc.kind == "ExternalOutput":
            out_names.append(name)
            shape = tuple(alloc.tensor_shape)
            dtype = mybir.dt.np(alloc.dtype)
            out_avals.append(jax.core.ShapedArray(shape, dtype))
            zero_outs.append(np.zeros(shape, dtype))
    n_params = len(in_names)
    all_in_names = in_names + out_names
    if partition_name is not None:
        all_in_names = all_in_names + [partition_name]

    def _bass_body(*args):
        operands = list(args)
        if partition_name is not None:
            operands.append(partition_id_tensor())
        outs = _bass_exec_p.bind(
            *operands,
            out_avals=tuple(out_avals),
            in_names=tuple(all_in_names),
            out_names=tuple(out_names),
            lowering_input_output_aliases=(),
            sim_require_finite=True,
            sim_require_nnan=True,
            nc=nc,
        )
        return tuple(outs)

    devices = jax.devices()[:NCORES]
    mesh = Mesh(np.asarray(devices), ("core",))
    spec = PartitionSpec("core")
    rspec = PartitionSpec()          # replicated (weights/biases)
    sharding = NamedSharding(mesh, spec)
    rsharding = NamedSharding(mesh, rspec)
    n_outs = len(out_names)
    # xT is per-core data; everything else is identical across cores
    in_specs = tuple(spec if nm == "xT" else rspec for nm in in_names)
    jitted = jax.jit(
        shard_map(
            _bass_body, mesh=mesh,
            in_specs=in_specs + (spec,) * n_outs,
            out_specs=(spec,) * n_outs,
            check_rep=False,
        ),
        keep_unused=True,
    )
    zeros_dev = [
        jax.device_put(np.concatenate([z] * NCORES, axis=0), sharding)
        for z in zero_outs
    ]
    runner = {
        "jitted": jitted, "in_names": in_names, "out_names": out_names,
        "sharding": sharding, "rsharding": rsharding,
        "zeros_dev": zeros_dev, "jax": jax,
    }
    _CACHE[key] = runner
    return runner


def _prep_inputs(x, Wq, bq, Wk, bk, Wv, bv, Wo, bo):
    """Host-side prep: arrays keyed by NEFF input name. xT is per-core
    concatenated; weights/biases are single copies (replicated spec)."""
    x = np.asarray(x, dtype=np.float32)
    Wq, Wk, Wv, Wo = (np.asarray(w, dtype=np.float32) for w in (Wq, Wk, Wv, Wo))
    bq, bk, bv, bo = (np.asarray(v, dtype=np.float32) for v in (bq, bk, bv, bo))

    xcat = np.ascontiguousarray(
        x.reshape(NCORES, T, E).transpose(0, 2, 1)).reshape(NCORES * E, T)
    # [H, 128, KT, D]: per-head slices DMA with 2304B-contiguous rows
    wqh = np.ascontiguousarray(
        Wq.reshape(KT, 128, H, D).transpose(2, 1, 0, 3))
    wkh = np.ascontiguousarray(
        Wk.reshape(KT, 128, H, D).transpose(2, 1, 0, 3))
    bqk = np.ascontiguousarray(
        np.concatenate([bq.reshape(H, D).T, bk.reshape(H, D).T], axis=1))

    return {
        "xT": xcat,
        "wqh": wqh, "wkh": wkh, "wv": Wv, "wo": Wo,
        "bqk": bqk, "bv1": np.ascontiguousarray(bv.reshape(1, E)),
        "bo1": np.ascontiguousarray(bo.reshape(1, E)),
    }


def _run(inputs, device_resident=None, with_bias=False):
    r = _get_runner(with_bias)
    args = []
    for name in r["in_names"]:
        if device_resident is not None and name in device_resident:
            args.append(device_resident[name])
        else:
            args.append(inputs[name])
    outs = r["jitted"](*args, *r["zeros_dev"])
    return {name: outs[i] for i, name in enumerate(r["out_names"])}


def _weights_on_device(inputs, with_bias=False):
    """device_put the (replicated) weight/bias arrays once per unique value."""
    import hashlib
    r = _get_runner(with_bias)
    key = hashlib.sha1()
    for name in sorted(inputs):
        if name == "xT":
            continue
        a = inputs[name]
        key.update(name.encode())
        key.update(a.shape.__repr__().encode())
        key.update(a.tobytes())
    key = key.hexdigest()
    cached = _CACHE.get("weights_dev")
    if cached is not None and cached[0] == key:
        return cached[1]
    dev = {
        name: r["jax"].device_put(a, r["rsharding"])
        for name, a in inputs.items() if name != "xT"
    }
    _CACHE["weights_dev"] = (key, dev)
    return dev


def kernel(x, Wq, bq, Wk, bk, Wv, bv, Wo, bo):
    with_bias = any(
        np.any(np.asarray(v)) for v in (bq, bk, bv, bo))
    inputs = _prep_inputs(x, Wq, bq, Wk, bk, Wv, bv, Wo, bo)
    dev = _weights_on_device(inputs, with_bias)
    outs = _run(inputs, dev, with_bias)
    out = np.asarray(outs["out"])          # [NCORES*T, E]
    return out.reshape(B, N, E)


def bench(x, Wq, bq, Wk, bk, Wv, bv, Wo, bo, iters=20):
    """Time repeated executions with all inputs device-resident.

    Returns (per_call_seconds, overhead_floor_seconds)."""
    import time
    r = _get_runner()
    inputs = _prep_inputs(x, Wq, bq, Wk, bk, Wv, bv, Wo, bo)
    dev = _weights_on_device(inputs)
    dev = dict(dev)
    dev["xT"] = r["jax"].device_put(inputs["xT"], r["sharding"])

    out = _run(inputs, dev)
    list(out.values())[0].block_until_ready()

    t0 = time.time()
    last = None
    for _ in range(iters):
        last = _run(inputs, dev)
    for v in last.values():
        v.block_until_ready()
    dt = (time.time() - t0) / iters
    return dt



# revision 65
# speedup vs baseline: 1.0823x; 1.0823x over previous
#!/usr/bin/env python3
"""Multi-head attention (B=16, N=1024, E=768, H=8, softmax-then-scale variant)
as a Bass/Tile kernel on 8 TRN2 NeuronCores, data-parallel over the batch.

Per core (2 batch elements, T=2048 tokens):
  - Q^T/K^T computed PACKED [768, N] per batch (6 full 128-row psum tiles,
    36864 PE rows vs 49152 per-head), then per-head [96, N] tiles are
    produced by partition-shifting SBUF->SBUF DMAs (the PE base-partition-0
    rule forbids slicing heads out of packed tiles directly).
  - energy^T per (b, h, qhalf, kt): fp32r matmul [128, 512], exp on ScalarE
    (no max subtraction: |energy| <~ 60 fits fp32/bf16 exp), exp output in
    bf16.
  - attn@V in NON-transposed z-form: z[q, d] accumulated per 128-token
    q-chunk with bf16 operands (8 kt matmuls of 97 cols each = 6208 PE rows
    per b,h vs 8192 transposed). Column 96 of Vhat is 1.0 so z[:, 96] is
    sumexp. Accumulation groups run sequentially (PSUM zero regions are
    bank-granular), ping-ponging across 2 psum banks.
  - normalize: z[:, 0:96] * (1/z[:, 96]) * (1/sqrt(E)) on DVE into bf16
    per-q-tile staging tiles [128, 768] (heads side by side).
  - staging tiles are transposed by the DMA xbar (dma_start_transpose,
    bf16) into packed zT [128, 6, N] so the output projection contracts
    over full 128-row tiles (36864 PE rows vs 49152), all-bf16 matmuls.
Software pipelining: attention units are emitted with a one-unit delay
(energy(u+1) before attn@V(u)) and PE filler (Vhat/projection/output work
of neighboring batches) is pumped between units to cover the ScalarE exp
latency.
"""
import os
import sys

sys.path.insert(0, "/opt/trn_rl_repo")

import numpy as np

B, N, E, H, D = 16, 1024, 768, 8, 96
NCORES = 8
BPC = B // NCORES          # batch elements per core
T = BPC * N                # tokens per core
KT = E // 128              # k-tiles over embedding dim (6)
NKT = N // 128             # k-tiles over sequence (8)

_CACHE = {}
_DEBUG = False

# per-head repack pieces: head h rows [96h, 96h+96) gathered from packed
# tiles t: list of (t, src_lo, src_hi, dst_lo)
def _repack_pieces(h):
    r0, r1 = 96 * h, 96 * h + 96
    out = []
    for t in range(KT):
        lo, hi = max(r0, 128 * t), min(r1, 128 * t + 128)
        if lo < hi:
            out.append((t, lo - 128 * t, hi - 128 * t, lo - r0))
    return out


def _build(with_bias=False):
    import concourse.tile as tile
    from concourse import bacc, mybir

    f32 = mybir.dt.float32
    f32r = mybir.dt.float32r
    bf16 = mybir.dt.bfloat16

    nc = bacc.Bacc("TRN2", target_bir_lowering=False, debug=False)

    # activation/weight fp32 inputs are declared float32r: the PE truncates
    # the mantissa on read, so feeding raw fp32 bits through DMA is
    # equivalent to an on-chip rounding pass (verified on HW)
    xT_d = nc.dram_tensor("xT", [E, T], f32r, kind="ExternalInput").ap()
    wq_d = nc.dram_tensor("wq", [E, E], f32r, kind="ExternalInput").ap()
    wk_d = nc.dram_tensor("wk", [E, E], f32r, kind="ExternalInput").ap()
    wv_d = nc.dram_tensor("wv", [E, E], f32r, kind="ExternalInput").ap()
    wo_d = nc.dram_tensor("wob", [E, E], bf16, kind="ExternalInput").ap()
    bqk_d = nc.dram_tensor("bqk", [128, 2 * KT], f32, kind="ExternalInput").ap()
    bv_d = nc.dram_tensor("bv1", [1, E], f32r, kind="ExternalInput").ap()
    bo_d = nc.dram_tensor("bo1", [1, E], bf16, kind="ExternalInput").ap()
    out_d = nc.dram_tensor("out", [T, E], f32, kind="ExternalOutput").ap()
    id_d = nc.dram_tensor("ident", [128, 128], bf16, kind="ExternalInput").ap()

    dbg = {}
    if _DEBUG:
        dbg["q1"] = nc.dram_tensor(
            "dbgq1", [D, N], f32, kind="ExternalOutput").ap()
        dbg["ex0"] = nc.dram_tensor(
            "dbgex0", [128, N], f32, kind="ExternalOutput").ap()
        dbg["st0"] = nc.dram_tensor(
            "dbgst0", [128, E], f32, kind="ExternalOutput").ap()
        dbg["zt0"] = nc.dram_tensor(
            "dbgzt0", [128, KT, 128], f32, kind="ExternalOutput").ap()
        dbg["wo0"] = nc.dram_tensor(
            "dbgwo0", [128, E], f32, kind="ExternalOutput").ap()
        dbg["zt1"] = nc.dram_tensor(
            "dbgzt1", [128, KT, 128], f32, kind="ExternalOutput").ap()


    with tile.TileContext(nc) as tc:
        _body(nc, tc, mybir,
              xT_d, wq_d, wk_d, wv_d, wo_d, bqk_d, bv_d, bo_d, out_d,
              id_d, with_bias, dbg)

    nc.compile()
    return nc


def _body(nc, tc, mybir,
          xT_d, wq_d, wk_d, wv_d, wo_d, bqk_d, bv_d, bo_d, out_d,
          id_d, with_bias, dbg=None):
    dbg = dbg or {}
    from contextlib import ExitStack

    f32 = mybir.dt.float32
    f32r = mybir.dt.float32r
    bf16 = mybir.dt.bfloat16
    Exp = mybir.ActivationFunctionType.Exp
    MULT = mybir.AluOpType.mult
    ADD = mybir.AluOpType.add
    DIV = mybir.AluOpType.divide
    INV_SQRT_E = float(np.float32(1.0 / np.sqrt(np.float32(E))))

    ctx = ExitStack()
    with ctx:
        persist = ctx.enter_context(tc.tile_pool(name="persist", bufs=1))
        # packed Q^T/K^T production tiles (per (tensor, t)), freed after
        # their per-head repack DMAs are consumed
        packp = ctx.enter_context(
            tc.tile_pool(name="packp", bufs=4 if not dbg else 3))
        # per-head [96, N] repacked q/k tiles
        headp = ctx.enter_context(tc.tile_pool(name="headp", bufs=2))
        # psum pools: shared (vhat/proj/final) 2 + energy 3 + z 2 = 7 banks
        shp = ctx.enter_context(tc.tile_pool(name="shp", bufs=2, space="PSUM"))
        epp = ctx.enter_context(tc.tile_pool(name="epp", bufs=2, space="PSUM"))
        zpp = ctx.enter_context(tc.tile_pool(name="zpp", bufs=2, space="PSUM"))

        xt = []
        vhat = []
        state = {}

        # ---------------- phase 0: loads ----------------
        # x is single-buffered per batch: [128, N] per e-chunk, batch 1
        # reloads over batch 0 once the b0 projections are all emitted
        for c in range(KT):
            xtc = persist.tile([128, N], f32r, name=f"xt{c}", tag=f"xt{c}")
            xt.append(xtc)

        def load_x_cols(b, lo, hi):
            for c in range(KT):
                nc.sync.dma_start(
                    out=xt[c][:, lo:hi],
                    in_=xT_d[c * 128:(c + 1) * 128, b * N + lo:b * N + hi])

        wvp = ctx.enter_context(tc.tile_pool(name="wvpool", bufs=1))
        wv = []
        # first x chunk interleaved with Wv so Vhat(0) can start early
        for c in range(KT):
            nc.sync.dma_start(
                out=xt[c][:, 0:256], in_=xT_d[c * 128:(c + 1) * 128, 0:256])
            wvc = wvp.tile([128, E], f32r, name=f"wv{c}", tag=f"wv{c}")
            nc.gpsimd.dma_start(out=wvc, in_=wv_d[c * 128:(c + 1) * 128, :])
            wv.append(wvc)

        # Q/K projection weights: loaded in [128, 128] blocks, t-major, so
        # the packed projection of tile t can start after only t+1 column
        # blocks have arrived (the phase-0 DMA stream is the serialized
        # bottleneck): wq blocks on SP, wk blocks on the gpsimd swdge
        # queue. wo is needed only late; its load is deferred into the
        # filler stream.
        wqt, wkt, wot = [], [], []
        for c in range(KT):
            w1 = persist.tile([128, E], f32r, name=f"wq{c}", tag=f"wq{c}")
            wqt.append(w1)
            w2 = persist.tile([128, E], f32r, name=f"wk{c}", tag=f"wk{c}")
            wkt.append(w2)

        def load_w_pair(p):
            """Load wq/wk columns [256p, 256p+256) (tiles t=2p, 2p+1):
            wq on SP, wk on the gpsimd swdge queue."""
            cols = slice(p * 256, (p + 1) * 256)
            for c in range(KT):
                nc.sync.dma_start(
                    out=wqt[c][:, cols],
                    in_=wq_d[c * 128:(c + 1) * 128, cols])
            for c in range(KT):
                nc.gpsimd.dma_start(
                    out=wkt[c][:, cols],
                    in_=wk_d[c * 128:(c + 1) * 128, cols])

        load_x_cols(0, 256, 512)
        load_x_cols(0, 512, 1024)
        load_w_pair(0)

        def load_wo():
            for t in range(KT):
                w3 = persist.tile([128, E], bf16, name=f"wo{t}", tag=f"wo{t}")
                nc.gpsimd.dma_start(out=w3, in_=wo_d[t * 128:(t + 1) * 128, :])
                wot.append(w3)
            idt = persist.tile([128, 128], bf16, name="ident", tag="ident")
            nc.gpsimd.dma_start(out=idt, in_=id_d)
            state["ident"] = idt
            tap("wo0", wot[0])

        if with_bias:
            bqk_t = persist.tile([128, 2 * KT], f32, name="bqk_t", tag="bqk_t")
            nc.gpsimd.dma_start(out=bqk_t, in_=bqk_d)
            bvr = persist.tile([1, E], f32r, name="bvr", tag="bvr")
            nc.gpsimd.dma_start(out=bvr, in_=bv_d)
            bob = persist.tile([1, E], bf16, name="bob", tag="bob")
            nc.gpsimd.dma_start(out=bob, in_=bo_d)
            ones_f = persist.tile([1, 128], f32, name="ones_f", tag="ones_f")
            nc.vector.memset(ones_f, 1.0)
            ones_r = persist.tile([1, 128], f32r, name="ones_r", tag="ones_r")
            nc.vector.tensor_copy(out=ones_r, in_=ones_f)
            ones_b = persist.tile([1, 128], bf16, name="ones_b", tag="ones_b")
            nc.vector.tensor_copy(out=ones_b, in_=ones_f)
            state.update(bqk_t=bqk_t, bvr=bvr, bob=bob,
                         ones_r=ones_r, ones_b=ones_b)

        # ---------------- emitters ----------------
        def build_vhat(b, mt):
            """Vhat[b*NKT+mt]: [128 tokens, H, D+1] bf16; column D = 1.0."""
            gmt = b * NKT + mt
            vh = persist.tile([128, H, D + 1], bf16, name=f"vhat{gmt}",
                              tag=f"vhat{gmt}")
            for half in range(2):
                pv = shp.tile([128, 4 * D], f32, name="shp", tag="shp")
                cols = slice(half * 4 * D, (half + 1) * 4 * D)
                for c in range(KT):
                    nc.tensor.matmul(
                        pv, xt[c][:, mt * 128:(mt + 1) * 128], wv[c][:, cols],
                        start=(c == 0),
                        stop=(not with_bias and c == KT - 1),
                    )
                if with_bias:
                    nc.tensor.matmul(
                        pv, state["ones_r"], state["bvr"][:, cols],
                        start=False, stop=True,
                    )
                nc.vector.tensor_copy(
                    out=vh[:, half * 4:(half + 1) * 4, 0:D],
                    in_=pv.rearrange("p (h d) -> p h d", h=4),
                )
            nc.vector.memset(vh[:, :, D:D + 1], 1.0)
            vhat.append(vh)
            if gmt == 0:
                tap("vh0", vh)

        heads = {}          # (b, 'q'|'k', h) -> [96, N] tile

        def tap(key, ap):
            """Debug: convert to f32 and ship to a debug DRAM tensor."""
            if key not in dbg:
                return
            shape = list(ap.shape)
            tmp = persist.tile(shape, f32, name=f"tap{key}", tag=f"tap{key}")
            nc.vector.tensor_copy(out=tmp, in_=ap)
            nc.sync.dma_start(out=dbg[key], in_=tmp)

        def qk_pack_chunk(b, ti, t, qc):
            """One packed projection chunk: psum [128, 512] accumulated over
            6 x-chunks -> packed tile (ti, t) columns qc*512:(qc+1)*512."""
            nm, wt = (("q", wqt), ("k", wkt))[ti]
            key = ("pk", b, ti, t)
            if key not in state:
                state[key] = packp.tile(
                    [128, N], f32r, name=f"pk{nm}{t}", tag="pk")
            pq = shp.tile([128, 512], f32, name="shp", tag="shp")
            for c in range(KT):
                nc.tensor.matmul(
                    pq,
                    wt[c][:, t * 128:(t + 1) * 128],
                    xt[c][:, qc * 512:(qc + 1) * 512],
                    start=(c == 0), stop=(c == KT - 1),
                )
            dst = state[key][:, qc * 512:(qc + 1) * 512]
            if with_bias:
                nc.vector.tensor_scalar(
                    out=dst, in0=pq,
                    scalar1=state["bqk_t"][:, ti * KT + t:ti * KT + t + 1],
                    scalar2=None, op0=ADD,
                )
            else:
                nc.vector.tensor_copy(out=dst, in_=pq)

        def qk_repack(b, ti, h):
            """Partition-shifting DMAs: packed tiles -> per-head [96, N]."""
            nm = ("q", "k")[ti]
            ht = headp.tile([D, N], f32r, name=f"h{nm}{h}", tag=f"h{nm}")
            for (t, slo, shi, dlo) in _repack_pieces(h):
                src = state[("pk", b, ti, t)]
                nc.sync.dma_start(
                    out=ht[dlo:dlo + (shi - slo), :], in_=src[slo:shi, :])
            heads[(b, nm, h)] = ht
            if b == 0 and ti == 0 and h == 1:
                tap("q1", ht)

        exps = {}           # unit -> list of 8 expT tiles

        def energy_kt(u, kt):
            """Energy + exp for one kt row: psum [128, 1024] in two
            512-col groups (separate banks), one bf16 exp over both."""
            b, h = u
            kh = heads[(b, "k", h)]
            qht = heads[(b, "q", h)]
            ep = epp.tile([128, N], f32, name="ep", tag="ep")
            for qc in range(2):
                nc.tensor.matmul(
                    ep[:, qc * 512:(qc + 1) * 512],
                    kh[:, kt * 128:(kt + 1) * 128],
                    qht[:, qc * 512:(qc + 1) * 512],
                    start=True, stop=True,
                )
            ex = expp.tile([128, N], bf16, name="ex", tag="ex")
            nc.scalar.activation(out=ex, in_=ep, func=Exp)
            exps.setdefault(u, []).append(ex)
            if u == (0, 0) and kt == 0:
                tap("ex0", ex)

        def attnv_j(u, jj):
            """One z accumulation group (q-token tile jj) + normalize."""
            b, h = u
            lst = exps[u]
            z = zpp.tile([128, D + 1], f32, name="z", tag="z")
            for kt in range(NKT):
                nc.tensor.matmul(
                    z,
                    lst[kt][:, jj * 128:(jj + 1) * 128],
                    vhat[b * NKT + kt][:, h, :],
                    start=(kt == 0), stop=(kt == NKT - 1),
                )
            # normalize: (z * (1/sumexp)) * (1/sqrt(E)) -> bf16 staging
            # (tensor_scalar divide fails the neuronxcc ISA check, so the
            # reciprocal stays a separate DVE op)
            rc = rcp.tile([128, 1], f32, name="rc", tag="rc")
            nc.vector.reciprocal(out=rc, in_=z[:, D:D + 1])
            nc.vector.tensor_scalar(
                out=staging[jj][:, h * D:(h + 1) * D],
                in0=z[:, 0:D], scalar1=rc, scalar2=INV_SQRT_E,
                op0=MULT, op1=MULT,
            )
            if h == H - 1:
                if b == 0 and jj == 0:
                    tap("st0", staging[jj])
                # transpose staging[jj] -> packed zT on the PE (the DMA
                # xbar transpose corrupts data under concurrency on this
                # HW path): per 128-col chunk, an is_transpose matmul with
                # a bf16 identity into psum, copied out on DVE mid-run /
                # ScalarE in the batch-1 tail (exp is done there)
                for t in range(KT):
                    tp = zpp.tile([128, 128], bf16, name="tp", tag="z")
                    nc.tensor.matmul(
                        tp, staging[jj][:, t * 128:(t + 1) * 128],
                        state["ident"], is_transpose=True,
                        start=True, stop=True)
                    if b == 1:
                        nc.scalar.copy(out=ztp[jj][:, t, :], in_=tp)
                    else:
                        nc.vector.tensor_copy(out=ztp[jj][:, t, :], in_=tp)
                if b == 0 and jj == 0:
                    tap("zt0", ztp[jj])
                if b == 0 and jj == 1:
                    tap("zt1", ztp[jj])
            if jj == NKT - 1:
                exps.pop(u)

        def final_group(b, jj, half):
            pr = shp.tile([128, 384], f32, name="shp", tag="shp")
            cols = slice(half * 384, (half + 1) * 384)
            for t in range(KT):
                nc.tensor.matmul(
                    pr,
                    ztp[jj][:, t, :],
                    wot[t][:, cols],
                    start=(t == 0),
                    stop=(not with_bias and t == KT - 1),
                )
            if with_bias:
                nc.tensor.matmul(
                    pr, state["ones_b"], state["bob"][:, cols],
                    start=False, stop=True,
                )
            ro = rop.tile([128, 384], f32, name="ro", tag="ro")
            # batch-1 groups run in the tail where ScalarE is idle (exp is
            # done); keeping them off DVE avoids head-of-line blocking of
            # the last normalize chain
            if b == 1:
                nc.scalar.copy(out=ro, in_=pr)
            else:
                nc.vector.tensor_copy(out=ro, in_=pr)
            tok0 = b * N
            nc.sync.dma_start(
                out=out_d[tok0 + jj * 128:tok0 + (jj + 1) * 128, cols],
                in_=ro)

        # pools for the attention phase
        # debug taps cost ~17KB of persist space; shrink pools to fit
        expp = ctx.enter_context(
            tc.tile_pool(name="expp", bufs=16 if not dbg else 8))
        rcp = ctx.enter_context(tc.tile_pool(name="rcp", bufs=4))
        rop = ctx.enter_context(tc.tile_pool(name="rop", bufs=5))
        stpool = ctx.enter_context(tc.tile_pool(name="stpool", bufs=1))
        ztpool = ctx.enter_context(tc.tile_pool(name="ztpool", bufs=1))

        staging = [
            stpool.tile([128, E], bf16, name=f"st{jj}", tag=f"st{jj}")
            for jj in range(NKT)
        ]
        # packed z^T, one tile per q-token tile jj so transpose writes and
        # final-projection reads get exact (non-bounding-box) deps
        ztp = [
            ztpool.tile([128, KT, 128], bf16, name=f"ztp{jj}", tag=f"ztp{jj}")
            for jj in range(NKT)
        ]

        # ---------------- phase 0 compute: vhat(b0) + qk(b0) t0/t1 --------
        for mt in range(NKT):
            build_vhat(0, mt)

        def qk_t_step(b, t):
            """Packed proj chunks for tile t (both tensors) + repacks of
            heads whose last source tile is t."""
            for ti in range(2):
                for qc in range(2):
                    qk_pack_chunk(b, ti, t, qc)
            for h in range(H):
                if _repack_pieces(h)[-1][0] == t:
                    qk_repack(b, 0, h)
                    qk_repack(b, 1, h)

        qk_t_step(0, 0)
        qk_t_step(0, 1)
        # remaining projection-weight columns stream in behind the early
        # repacks while the first attention units run
        load_w_pair(1)
        load_w_pair(2)

        # ------------- main: global attention pipeline with fillers -------
        # batch-0 filler: remaining w-blocks + rest of qk(b0), then x(b1)
        # load, vhat(b1), wo load, qk(b1) pack chunks (repacks deferred to
        # just-in-time pre-hooks). qk(b0) chunks must all be emitted
        # before the x(b1) load (single-buffered x; tile deps follow
        # emission order).
        def fitem(*fns):
            def run():
                for f in fns:
                    f()
            return run

        fill0 = [
            fitem(lambda: qk_t_step(0, 2)),
            fitem(lambda: qk_t_step(0, 3)),
            fitem(lambda: qk_t_step(0, 4)),
            fitem(lambda: qk_t_step(0, 5), lambda: load_x_cols(1, 0, N)),
        ]
        fill0.extend(
            lambda mt=mt: build_vhat(1, mt) for mt in range(NKT))
        fill0.append(load_wo)
        fill0.extend(
            lambda t=t: [qk_pack_chunk(1, ti, t, qc)
                         for ti in range(2) for qc in range(2)]
            for t in range(KT))

        def first_b1_repacks():
            for hh in (0, 1):
                qk_repack(1, 0, hh)
                qk_repack(1, 1, hh)
        fill0.append(first_b1_repacks)
        fin0 = [
            lambda jj=jj, half=half: final_group(0, jj, half)
            for jj in range(NKT) for half in range(2)
        ]
        fills = {0: fill0, 1: fin0}

        units = [(b, h) for b in range(BPC) for h in range(H)]
        prev = None
        for i, u in enumerate(units):
            b, h = u
            if b == 1 and h == 0:
                # drain remaining b0 fillers (qk(b1) packs among them),
                # then emit the first two b1 head repacks
                for f in fills[0]:
                    f()
                fills[0] = []
                for hh in (0, 1):
                    qk_repack(1, 0, hh)
                    qk_repack(1, 1, hh)
            if b == 1 and h + 2 < H:
                # just-in-time repack: its head-pool WAR wait (energy of
                # head h) is nearly satisfied at emission time
                qk_repack(1, 0, h + 2)
                qk_repack(1, 1, h + 2)
            if b == 0:
                # force-drain fillers until this unit's heads are repacked
                while (0, "q", h) not in heads:
                    fills[0].pop(0)()
            # kt-granular interleave: energy(u, kt) | attnv(prev, j=kt) |
            # filler, so the PE never sits behind a not-yet-computed exp
            # and the ScalarE pipeline stays fed
            fill = fills[b]
            budget = -(-len(fill) // max(1, H - h))
            emitted = 0
            for kt in range(NKT):
                energy_kt(u, kt)
                if prev is not None:
                    attnv_j(prev, kt)
                if emitted < budget and fill and kt % 2 == 1:
                    fill.pop(0)()
                    emitted += 1
            while emitted < budget and fill:
                fill.pop(0)()
                emitted += 1
            prev = u

        # last unit's attn@V (emits the per-j dma transposes), then final(b1)
        for jj in range(NKT):
            attnv_j(prev, jj)
        for f in fills[1]:
            f()
        for jj in range(NKT):
            for half in range(2):
                final_group(1, jj, half)


def _get_runner(with_bias=False):
    """Build (once per variant) a jitted shard_map executing the NEFF."""
    key = ("runner", with_bias)
    if key in _CACHE:
        return _CACHE[key]

    import jax
    from jax.experimental.shard_map import shard_map
    from jax.sharding import Mesh, NamedSharding, PartitionSpec
    from concourse import mybir
    from concourse.bass2jax import (
        _bass_exec_p, install_neuronx_cc_hook, partition_id_tensor)

    nc = _build(with_bias=with_bias)
    install_neuronx_cc_hook()

    partition_name = (
        nc.partition_id_tensor.name if nc.partition_id_tensor else None)
    in_names, out_names, out_avals, zero_outs = [], [], [], []
    for alloc in nc.m.functions[0].allocations:
        if not isinstance(alloc, mybir.MemoryLocationSet):
            continue
        name = alloc.memorylocations[0].name
        if alloc.kind == "ExternalInput":
            if name != partition_name:
                in_names.append(name)
        elif alloc.kind == "ExternalOutput":
            out_names.append(name)
            shape = tuple(alloc.tensor_shape)
            dtype = mybir.dt.np(alloc.dtype)
            out_avals.append(jax.core.ShapedArray(shape, dtype))
            zero_outs.append(np.zeros(shape, dtype))
    n_params = len(in_names)
    all_in_names = in_names + out_names
    if partition_name is not None:
        all_in_names = all_in_names + [partition_name]

    def _bass_body(*args):
        operands = list(args)
        if partition_name is not None:
            operands.append(partition_id_tensor())
        outs = _bass_exec_p.bind(
            *operands,
            out_avals=tuple(out_avals),
            in_names=tuple(all_in_names),
            out_names=tuple(out_names),
            lowering_input_output_aliases=(),
            sim_require_finite=True,
            sim_require_nnan=True,
            nc=nc,
        )
        return tuple(outs)

    devices = jax.devices()[:NCORES]
    mesh = Mesh(np.asarray(devices), ("core",))
    spec = PartitionSpec("core")
    rspec = PartitionSpec()          # replicated (weights/biases)
    sharding = NamedSharding(mesh, spec)
    rsharding = NamedSharding(mesh, rspec)
    n_outs = len(out_names)
    # xT is per-core data; everything else is identical across cores
    in_specs = tuple(spec if nm == "xT" else rspec for nm in in_names)
    jitted = jax.jit(
        shard_map(
            _bass_body, mesh=mesh,
            in_specs=in_specs + (spec,) * n_outs,
            out_specs=(spec,) * n_outs,
            check_rep=False,
        ),
        keep_unused=True,
    )
    zeros_dev = [
        jax.device_put(np.concatenate([z] * NCORES, axis=0), sharding)
        for z in zero_outs
    ]
    runner = {
        "jitted": jitted, "in_names": in_names, "out_names": out_names,
        "sharding": sharding, "rsharding": rsharding,
        "zeros_dev": zeros_dev, "jax": jax,
    }
    _CACHE[key] = runner
    return runner


def _prep_inputs(x, Wq, bq, Wk, bk, Wv, bv, Wo, bo):
    """Host-side prep: arrays keyed by NEFF input name. xT is per-core
    concatenated; weights/biases are single copies (replicated spec)."""
    import ml_dtypes
    x = np.asarray(x, dtype=np.float32)
    Wq, Wk, Wv, Wo = (np.asarray(w, dtype=np.float32) for w in (Wq, Wk, Wv, Wo))
    bq, bk, bv, bo = (np.asarray(v, dtype=np.float32) for v in (bq, bk, bv, bo))

    xcat = np.ascontiguousarray(
        x.reshape(NCORES, T, E).transpose(0, 2, 1)).reshape(NCORES * E, T)
    # bqk [128, 2*KT]: column (i*KT + t) = rows 128t..128t+128 of bq/bk
    bqk = np.concatenate(
        [bq.reshape(KT, 128).T, bk.reshape(KT, 128).T], axis=1)

    return {
        "xT": xcat,
        "wq": Wq, "wk": Wk, "wv": Wv,
        "wob": Wo.astype(ml_dtypes.bfloat16),
        "bqk": np.ascontiguousarray(bqk),
        "bv1": np.ascontiguousarray(bv.reshape(1, E)),
        "bo1": bo.reshape(1, E).astype(ml_dtypes.bfloat16),
        "ident": np.eye(128, dtype=ml_dtypes.bfloat16),
    }


def _run(inputs, device_resident=None, with_bias=False):
    r = _get_runner(with_bias)
    args = []
    for name in r["in_names"]:
        if device_resident is not None and name in device_resident:
            args.append(device_resident[name])
        else:
            args.append(inputs[name])
    outs = r["jitted"](*args, *r["zeros_dev"])
    return {name: outs[i] for i, name in enumerate(r["out_names"])}


def _weights_on_device(inputs, with_bias=False):
    """device_put the (replicated) weight/bias arrays once per unique value."""
    import hashlib
    r = _get_runner(with_bias)
    key = hashlib.sha1()
    for name in sorted(inputs):
        if name == "xT":
            continue
        a = inputs[name]
        key.update(name.encode())
        key.update(a.shape.__repr__().encode())
        key.update(a.tobytes())
    key = key.hexdigest()
    cached = _CACHE.get("weights_dev")
    if cached is not None and cached[0] == key:
        return cached[1]
    dev = {
        name: r["jax"].device_put(a, r["rsharding"])
        for name, a in inputs.items() if name != "xT"
    }
    _CACHE["weights_dev"] = (key, dev)
    return dev


def kernel(x, Wq, bq, Wk, bk, Wv, bv, Wo, bo):
    with_bias = any(
        np.any(np.asarray(v)) for v in (bq, bk, bv, bo))
    inputs = _prep_inputs(x, Wq, bq, Wk, bk, Wv, bv, Wo, bo)
    dev = _weights_on_device(inputs, with_bias)
    outs = _run(inputs, dev, with_bias)
    out = np.asarray(outs["out"])          # [NCORES*T, E]
    return out.reshape(B, N, E)


def bench(x, Wq, bq, Wk, bk, Wv, bv, Wo, bo, iters=20):
    """Time repeated executions with all inputs device-resident."""
    import time
    r = _get_runner()
    inputs = _prep_inputs(x, Wq, bq, Wk, bk, Wv, bv, Wo, bo)
    dev = _weights_on_device(inputs)
    dev = dict(dev)
    dev["xT"] = r["jax"].device_put(inputs["xT"], r["sharding"])

    out = _run(inputs, dev)
    list(out.values())[0].block_until_ready()

    t0 = time.time()
    last = None
    for _ in range(iters):
        last = _run(inputs, dev)
    for v in last.values():
        v.block_until_ready()
    dt = (time.time() - t0) / iters
    return dt


# revision 82
# speedup vs baseline: 1.0841x; 1.0016x over previous
#!/usr/bin/env python3
"""Multi-head attention (B=16, N=1024, E=768, H=8, softmax-then-scale variant)
as a Bass/Tile kernel on 8 TRN2 NeuronCores, data-parallel over the batch.

Per core (2 batch elements, T=2048 tokens):
  - Q^T/K^T computed PACKED [768, N] per batch (6 full 128-row psum tiles,
    36864 PE rows vs 49152 per-head), then per-head [96, N] tiles are
    produced by partition-shifting SBUF->SBUF DMAs (the PE base-partition-0
    rule forbids slicing heads out of packed tiles directly).
  - energy^T per (b, h, qhalf, kt): fp32r matmul [128, 512], exp on ScalarE
    (no max subtraction: |energy| <~ 60 fits fp32/bf16 exp), exp output in
    bf16.
  - attn@V in NON-transposed z-form: z[q, d] accumulated per 128-token
    q-chunk with bf16 operands (8 kt matmuls of 97 cols each = 6208 PE rows
    per b,h vs 8192 transposed). Column 96 of Vhat is 1.0 so z[:, 96] is
    sumexp. Accumulation groups run sequentially (PSUM zero regions are
    bank-granular), ping-ponging across 2 psum banks.
  - normalize: z[:, 0:96] * (1/z[:, 96]) * (1/sqrt(E)) on DVE into bf16
    per-q-tile staging tiles [128, 768] (heads side by side).
  - staging tiles are transposed by the DMA xbar (dma_start_transpose,
    bf16) into packed zT [128, 6, N] so the output projection contracts
    over full 128-row tiles (36864 PE rows vs 49152), all-bf16 matmuls.
Software pipelining: attention units are emitted with a one-unit delay
(energy(u+1) before attn@V(u)) and PE filler (Vhat/projection/output work
of neighboring batches) is pumped between units to cover the ScalarE exp
latency.
"""
import os
import sys

sys.path.insert(0, "/opt/trn_rl_repo")

import numpy as np

B, N, E, H, D = 16, 1024, 768, 8, 96
NCORES = 8
BPC = B // NCORES          # batch elements per core
T = BPC * N                # tokens per core
KT = E // 128              # k-tiles over embedding dim (6)
NKT = N // 128             # k-tiles over sequence (8)

_CACHE = {}
_DEBUG = False

# per-head repack pieces: head h rows [96h, 96h+96) gathered from packed
# tiles t: list of (t, src_lo, src_hi, dst_lo)
def _repack_pieces(h):
    r0, r1 = 96 * h, 96 * h + 96
    out = []
    for t in range(KT):
        lo, hi = max(r0, 128 * t), min(r1, 128 * t + 128)
        if lo < hi:
            out.append((t, lo - 128 * t, hi - 128 * t, lo - r0))
    return out


def _build(with_bias=False):
    import concourse.tile as tile
    from concourse import bacc, mybir

    f32 = mybir.dt.float32
    f32r = mybir.dt.float32r
    bf16 = mybir.dt.bfloat16

    nc = bacc.Bacc("TRN2", target_bir_lowering=False, debug=False)

    # activation/weight fp32 inputs are declared float32r: the PE truncates
    # the mantissa on read, so feeding raw fp32 bits through DMA is
    # equivalent to an on-chip rounding pass (verified on HW)
    xT_d = nc.dram_tensor("xT", [E, T], f32r, kind="ExternalInput").ap()
    wq_d = nc.dram_tensor("wq", [E, E], f32r, kind="ExternalInput").ap()
    wk_d = nc.dram_tensor("wk", [E, E], f32r, kind="ExternalInput").ap()
    wv_d = nc.dram_tensor("wv", [E, E], f32r, kind="ExternalInput").ap()
    wo_d = nc.dram_tensor("wob", [E, E], bf16, kind="ExternalInput").ap()
    bqk_d = nc.dram_tensor("bqk", [128, 2 * KT], f32, kind="ExternalInput").ap()
    bv_d = nc.dram_tensor("bv1", [1, E], f32r, kind="ExternalInput").ap()
    bo_d = nc.dram_tensor("bo1", [1, E], bf16, kind="ExternalInput").ap()
    out_d = nc.dram_tensor("out", [T, E], f32, kind="ExternalOutput").ap()
    id_d = nc.dram_tensor("ident", [128, 128], bf16, kind="ExternalInput").ap()

    dbg = {}
    if _DEBUG:
        dbg["q1"] = nc.dram_tensor(
            "dbgq1", [D, N], f32, kind="ExternalOutput").ap()
        dbg["ex0"] = nc.dram_tensor(
            "dbgex0", [128, N], f32, kind="ExternalOutput").ap()
        dbg["st0"] = nc.dram_tensor(
            "dbgst0", [128, E], f32, kind="ExternalOutput").ap()
        dbg["zt0"] = nc.dram_tensor(
            "dbgzt0", [128, KT, 128], f32, kind="ExternalOutput").ap()
        dbg["wo0"] = nc.dram_tensor(
            "dbgwo0", [128, E], f32, kind="ExternalOutput").ap()
        dbg["zt1"] = nc.dram_tensor(
            "dbgzt1", [128, KT, 128], f32, kind="ExternalOutput").ap()


    with tile.TileContext(nc) as tc:
        _body(nc, tc, mybir,
              xT_d, wq_d, wk_d, wv_d, wo_d, bqk_d, bv_d, bo_d, out_d,
              id_d, with_bias, dbg)

    nc.compile()
    return nc


def _body(nc, tc, mybir,
          xT_d, wq_d, wk_d, wv_d, wo_d, bqk_d, bv_d, bo_d, out_d,
          id_d, with_bias, dbg=None):
    dbg = dbg or {}
    from contextlib import ExitStack

    f32 = mybir.dt.float32
    f32r = mybir.dt.float32r
    bf16 = mybir.dt.bfloat16
    Exp = mybir.ActivationFunctionType.Exp
    MULT = mybir.AluOpType.mult
    ADD = mybir.AluOpType.add
    DIV = mybir.AluOpType.divide
    INV_SQRT_E = float(np.float32(1.0 / np.sqrt(np.float32(E))))

    ctx = ExitStack()
    with ctx:
        persist = ctx.enter_context(tc.tile_pool(name="persist", bufs=1))
        # packed Q^T/K^T production tiles (per (tensor, t)), freed after
        # their per-head repack DMAs are consumed
        packp = ctx.enter_context(
            tc.tile_pool(name="packp", bufs=4 if not dbg else 3))
        # per-head [96, N] repacked q/k tiles
        headp = ctx.enter_context(tc.tile_pool(name="headp", bufs=2))
        # psum pools: shared (vhat/proj/final) 2 + energy 3 + z 2 = 7 banks
        shp = ctx.enter_context(tc.tile_pool(name="shp", bufs=2, space="PSUM"))
        epp = ctx.enter_context(tc.tile_pool(name="epp", bufs=2, space="PSUM"))
        zpp = ctx.enter_context(tc.tile_pool(name="zpp", bufs=2, space="PSUM"))

        xt = []
        vhat = []
        state = {}

        # ---------------- phase 0: loads ----------------
        # x is single-buffered per batch: [128, N] per e-chunk, batch 1
        # reloads over batch 0 once the b0 projections are all emitted
        for c in range(KT):
            xtc = persist.tile([128, N], f32r, name=f"xt{c}", tag=f"xt{c}")
            xt.append(xtc)

        def load_x_cols(b, lo, hi):
            for c in range(KT):
                nc.sync.dma_start(
                    out=xt[c][:, lo:hi],
                    in_=xT_d[c * 128:(c + 1) * 128, b * N + lo:b * N + hi])

        wvp = ctx.enter_context(tc.tile_pool(name="wvpool", bufs=1))
        wv = []
        # first x chunk interleaved with the first Wv column-half so the
        # Vhat half-0 sweep can start after ~1.6MB of DMA
        for c in range(KT):
            nc.sync.dma_start(
                out=xt[c][:, 0:128], in_=xT_d[c * 128:(c + 1) * 128, 0:128])
            wvc = wvp.tile([128, E], f32r, name=f"wv{c}", tag=f"wv{c}")
            nc.gpsimd.dma_start(
                out=wvc[:, 0:4 * D], in_=wv_d[c * 128:(c + 1) * 128, 0:4 * D])
            wv.append(wvc)

        def load_wv_half1():
            for c in range(KT):
                nc.gpsimd.dma_start(
                    out=wv[c][:, 4 * D:E],
                    in_=wv_d[c * 128:(c + 1) * 128, 4 * D:E])

        # Q/K projection weights: loaded in [128, 128] blocks, t-major, so
        # the packed projection of tile t can start after only t+1 column
        # blocks have arrived (the phase-0 DMA stream is the serialized
        # bottleneck): wq blocks on SP, wk blocks on the gpsimd swdge
        # queue. wo is needed only late; its load is deferred into the
        # filler stream.
        wqt, wkt, wot = [], [], []
        for c in range(KT):
            w1 = persist.tile([128, E], f32r, name=f"wq{c}", tag=f"wq{c}")
            wqt.append(w1)
            w2 = persist.tile([128, E], f32r, name=f"wk{c}", tag=f"wk{c}")
            wkt.append(w2)

        def load_w_pair(p):
            """Load wq/wk columns [256p, 256p+256) (tiles t=2p, 2p+1):
            wq on SP, wk on the gpsimd swdge queue."""
            cols = slice(p * 256, (p + 1) * 256)
            for c in range(KT):
                nc.sync.dma_start(
                    out=wqt[c][:, cols],
                    in_=wq_d[c * 128:(c + 1) * 128, cols])
            for c in range(KT):
                nc.gpsimd.dma_start(
                    out=wkt[c][:, cols],
                    in_=wk_d[c * 128:(c + 1) * 128, cols])

        idt = persist.tile([128, 128], bf16, name="ident", tag="ident")
        nc.gpsimd.dma_start(out=idt, in_=id_d)
        state["ident"] = idt
        load_x_cols(0, 128, 512)
        load_w_pair(0)
        load_x_cols(0, 512, 1024)
        load_wv_half1()

        def load_wo():
            for t in range(KT):
                w3 = persist.tile([128, E], bf16, name=f"wo{t}", tag=f"wo{t}")
                nc.gpsimd.dma_start(out=w3, in_=wo_d[t * 128:(t + 1) * 128, :])
                wot.append(w3)
            tap("wo0", wot[0])

        if with_bias:
            bqk_t = persist.tile([128, 2 * KT], f32, name="bqk_t", tag="bqk_t")
            nc.gpsimd.dma_start(out=bqk_t, in_=bqk_d)
            bvr = persist.tile([1, E], f32r, name="bvr", tag="bvr")
            nc.gpsimd.dma_start(out=bvr, in_=bv_d)
            bob = persist.tile([1, E], bf16, name="bob", tag="bob")
            nc.gpsimd.dma_start(out=bob, in_=bo_d)
            ones_f = persist.tile([1, 128], f32, name="ones_f", tag="ones_f")
            nc.vector.memset(ones_f, 1.0)
            ones_r = persist.tile([1, 128], f32r, name="ones_r", tag="ones_r")
            nc.vector.tensor_copy(out=ones_r, in_=ones_f)
            ones_b = persist.tile([1, 128], bf16, name="ones_b", tag="ones_b")
            nc.vector.tensor_copy(out=ones_b, in_=ones_f)
            state.update(bqk_t=bqk_t, bvr=bvr, bob=bob,
                         ones_r=ones_r, ones_b=ones_b)

        # ---------------- emitters ----------------
        def build_vhat_half(b, mt, half):
            """One half (4 heads) of Vhat[b*NKT+mt]; creates the tile and
            its 1.0 sumexp column on half 0."""
            gmt = b * NKT + mt
            if half == 0:
                vh = persist.tile([128, H, D + 1], bf16, name=f"vhat{gmt}",
                                  tag=f"vhat{gmt}")
                nc.vector.memset(vh[:, :, D:D + 1], 1.0)
                state[("vh", gmt)] = vh
                vhat.append(vh)
            vh = state[("vh", gmt)]
            pv = shp.tile([128, 4 * D], f32, name="shp", tag="shp")
            cols = slice(half * 4 * D, (half + 1) * 4 * D)
            for c in range(KT):
                nc.tensor.matmul(
                    pv, xt[c][:, mt * 128:(mt + 1) * 128], wv[c][:, cols],
                    start=(c == 0),
                    stop=(not with_bias and c == KT - 1),
                )
            if with_bias:
                nc.tensor.matmul(
                    pv, state["ones_r"], state["bvr"][:, cols],
                    start=False, stop=True,
                )
            nc.vector.tensor_copy(
                out=vh[:, half * 4:(half + 1) * 4, 0:D],
                in_=pv.rearrange("p (h d) -> p h d", h=4),
            )

        def build_vhat(b, mt):
            build_vhat_half(b, mt, 0)
            build_vhat_half(b, mt, 1)

        heads = {}          # (b, 'q'|'k', h) -> [96, N] tile

        def tap(key, ap):
            """Debug: convert to f32 and ship to a debug DRAM tensor."""
            if key not in dbg:
                return
            shape = list(ap.shape)
            tmp = persist.tile(shape, f32, name=f"tap{key}", tag=f"tap{key}")
            nc.vector.tensor_copy(out=tmp, in_=ap)
            nc.sync.dma_start(out=dbg[key], in_=tmp)

        def qk_pack_chunk(b, ti, t, qc):
            """One packed projection chunk: psum [128, 512] accumulated over
            6 x-chunks -> packed tile (ti, t) columns qc*512:(qc+1)*512."""
            nm, wt = (("q", wqt), ("k", wkt))[ti]
            key = ("pk", b, ti, t)
            if key not in state:
                state[key] = packp.tile(
                    [128, N], f32r, name=f"pk{nm}{t}", tag="pk")
            pq = shp.tile([128, 512], f32, name="shp", tag="shp")
            for c in range(KT):
                nc.tensor.matmul(
                    pq,
                    wt[c][:, t * 128:(t + 1) * 128],
                    xt[c][:, qc * 512:(qc + 1) * 512],
                    start=(c == 0), stop=(c == KT - 1),
                )
            dst = state[key][:, qc * 512:(qc + 1) * 512]
            if with_bias:
                nc.vector.tensor_scalar(
                    out=dst, in0=pq,
                    scalar1=state["bqk_t"][:, ti * KT + t:ti * KT + t + 1],
                    scalar2=None, op0=ADD,
                )
            else:
                nc.vector.tensor_copy(out=dst, in_=pq)

        def qk_repack(b, ti, h):
            """Partition-shifting DMAs: packed tiles -> per-head [96, N]."""
            nm = ("q", "k")[ti]
            ht = headp.tile([D, N], f32r, name=f"h{nm}{h}", tag=f"h{nm}")
            for (t, slo, shi, dlo) in _repack_pieces(h):
                src = state[("pk", b, ti, t)]
                nc.sync.dma_start(
                    out=ht[dlo:dlo + (shi - slo), :], in_=src[slo:shi, :])
            heads[(b, nm, h)] = ht
            if b == 0 and ti == 0 and h == 1:
                tap("q1", ht)

        exps = {}           # unit -> list of 8 expT tiles

        def energy_kt(u, kt):
            """Energy + exp for one kt row: psum [128, 1024] in two
            512-col groups (separate banks), one bf16 exp over both."""
            b, h = u
            kh = heads[(b, "k", h)]
            qht = heads[(b, "q", h)]
            ep = epp.tile([128, N], f32, name="ep", tag="ep")
            for qc in range(2):
                nc.tensor.matmul(
                    ep[:, qc * 512:(qc + 1) * 512],
                    kh[:, kt * 128:(kt + 1) * 128],
                    qht[:, qc * 512:(qc + 1) * 512],
                    start=True, stop=True,
                )
            ex = expp.tile([128, N], bf16, name="ex", tag="ex")
            nc.scalar.activation(out=ex, in_=ep, func=Exp)
            exps.setdefault(u, []).append(ex)
            if u == (0, 0) and kt == 0:
                tap("ex0", ex)

        def attnv_j(u, jj):
            """One z accumulation group (q-token tile jj) + normalize."""
            b, h = u
            lst = exps[u]
            z = zpp.tile([128, D + 1], f32, name="z", tag="z")
            for kt in range(NKT):
                nc.tensor.matmul(
                    z,
                    lst[kt][:, jj * 128:(jj + 1) * 128],
                    vhat[b * NKT + kt][:, h, :],
                    start=(kt == 0), stop=(kt == NKT - 1),
                )
            # normalize: (z * (1/sumexp)) * (1/sqrt(E)) -> bf16 staging
            # (tensor_scalar divide fails the neuronxcc ISA check, so the
            # reciprocal stays a separate DVE op)
            rc = rcp.tile([128, 1], f32, name="rc", tag="rc")
            nc.vector.reciprocal(out=rc, in_=z[:, D:D + 1])
            nc.vector.tensor_scalar(
                out=staging[jj][:, h * D:(h + 1) * D],
                in0=z[:, 0:D], scalar1=rc, scalar2=INV_SQRT_E,
                op0=MULT, op1=MULT,
            )
            # transpose staging[jj] -> packed zT on the PE (the DMA xbar
            # transpose corrupts data under concurrency on this HW path):
            # per 128-col chunk, an is_transpose matmul with a bf16
            # identity into psum, then copied out. For batch 0 planes
            # 0-2 (heads 0-3) transpose already at h3, spreading the DVE
            # copy load away from the batch transition; batch 1 keeps all
            # planes at h7 (its ztp writes must follow final(b0) reads).
            # Tail copies (b1) alternate ScalarE/DVE - exp is done there.
            if b == 0 and h == 3:
                trange = range(0, KT // 2)
            elif h == H - 1:
                trange = range(0, KT) if b == 1 else range(KT // 2, KT)
            else:
                trange = ()
            for t in trange:
                tp = zpp.tile([128, 128], bf16, name="tp", tag="z")
                nc.tensor.matmul(
                    tp, staging[jj][:, t * 128:(t + 1) * 128],
                    state["ident"], is_transpose=True,
                    start=True, stop=True)
                if b == 1 and t % 2 == 0:
                    nc.scalar.copy(out=ztp[jj][:, t, :], in_=tp)
                else:
                    nc.vector.tensor_copy(out=ztp[jj][:, t, :], in_=tp)
            if h == H - 1:
                if b == 0 and jj == 0:
                    tap("st0", staging[jj])
                    tap("zt0", ztp[jj])
                if b == 0 and jj == 1:
                    tap("zt1", ztp[jj])
            if jj == NKT - 1:
                exps.pop(u)

        def final_group(b, jj, half):
            pr = shp.tile([128, 384], f32, name="shp", tag="shp")
            cols = slice(half * 384, (half + 1) * 384)
            for t in range(KT):
                nc.tensor.matmul(
                    pr,
                    ztp[jj][:, t, :],
                    wot[t][:, cols],
                    start=(t == 0),
                    stop=(not with_bias and t == KT - 1),
                )
            if with_bias:
                nc.tensor.matmul(
                    pr, state["ones_b"], state["bob"][:, cols],
                    start=False, stop=True,
                )
            ro = rop.tile([128, 384], f32, name="ro", tag="ro")
            # batch-1 groups run in the tail where ScalarE is idle (exp is
            # done); keeping them off DVE avoids head-of-line blocking of
            # the last normalize chain
            if b == 1:
                nc.scalar.copy(out=ro, in_=pr)
            else:
                nc.vector.tensor_copy(out=ro, in_=pr)
            tok0 = b * N
            nc.sync.dma_start(
                out=out_d[tok0 + jj * 128:tok0 + (jj + 1) * 128, cols],
                in_=ro)

        # pools for the attention phase
        # debug taps cost ~17KB of persist space; shrink pools to fit
        expp = ctx.enter_context(
            tc.tile_pool(name="expp", bufs=16 if not dbg else 8))
        rcp = ctx.enter_context(tc.tile_pool(name="rcp", bufs=4))
        rop = ctx.enter_context(tc.tile_pool(name="rop", bufs=5))
        stpool = ctx.enter_context(tc.tile_pool(name="stpool", bufs=1))
        ztpool = ctx.enter_context(tc.tile_pool(name="ztpool", bufs=1))

        staging = [
            stpool.tile([128, E], bf16, name=f"st{jj}", tag=f"st{jj}")
            for jj in range(NKT)
        ]
        # packed z^T, one tile per q-token tile jj so transpose writes and
        # final-projection reads get exact (non-bounding-box) deps
        ztp = [
            ztpool.tile([128, KT, 128], bf16, name=f"ztp{jj}", tag=f"ztp{jj}")
            for jj in range(NKT)
        ]

        # ---------------- phase 0 compute: vhat(b0) + qk(b0) t0/t1 --------
        # half-0 sweep only: chases the x/wv-half0 DMA stream; half 1
        # lands as early filler inside the first attention unit, by which
        # time wv half 1 has arrived
        for mt in range(NKT):
            build_vhat_half(0, mt, 0)

        def qk_t_step(b, t):
            """Packed proj chunks for tile t (both tensors) + repacks of
            heads whose last source tile is t."""
            for ti in range(2):
                for qc in range(2):
                    qk_pack_chunk(b, ti, t, qc)
            for h in range(H):
                if _repack_pieces(h)[-1][0] == t:
                    qk_repack(b, 0, h)
                    qk_repack(b, 1, h)

        qk_t_step(0, 0)
        qk_t_step(0, 1)
        # remaining projection-weight columns stream in behind the early
        # repacks while the first attention units run
        load_w_pair(1)
        load_w_pair(2)

        # ------------- main: global attention pipeline with fillers -------
        # batch-0 filler: remaining w-blocks + rest of qk(b0), then x(b1)
        # load, vhat(b1), wo load, qk(b1) pack chunks (repacks deferred to
        # just-in-time pre-hooks). qk(b0) chunks must all be emitted
        # before the x(b1) load (single-buffered x; tile deps follow
        # emission order).
        def fitem(*fns):
            def run():
                for f in fns:
                    f()
            return run

        def set_vh1_done():
            state["vh1_0"] = True

        fill0 = [
            fitem(lambda: build_vhat_half(0, 0, 1),
                  lambda: build_vhat_half(0, 1, 1)),
            fitem(lambda: build_vhat_half(0, 2, 1),
                  lambda: build_vhat_half(0, 3, 1)),
            fitem(lambda: build_vhat_half(0, 4, 1),
                  lambda: build_vhat_half(0, 5, 1)),
            fitem(lambda: build_vhat_half(0, 6, 1),
                  lambda: build_vhat_half(0, 7, 1), set_vh1_done),
            fitem(lambda: qk_t_step(0, 2)),
            fitem(lambda: qk_t_step(0, 3)),
            fitem(lambda: qk_t_step(0, 4)),
            fitem(lambda: qk_t_step(0, 5), lambda: load_x_cols(1, 0, N)),
        ]
        fill0.extend(
            lambda mt=mt: build_vhat(1, mt) for mt in range(NKT))
        fill0.append(load_wo)
        fill0.extend(
            lambda t=t: [qk_pack_chunk(1, ti, t, qc)
                         for ti in range(2) for qc in range(2)]
            for t in range(KT))

        def first_b1_repacks():
            for hh in (0, 1):
                qk_repack(1, 0, hh)
                qk_repack(1, 1, hh)
        fill0.append(first_b1_repacks)
        fin0 = [
            lambda jj=jj, half=half: final_group(0, jj, half)
            for jj in range(NKT) for half in range(2)
        ]
        fills = {0: fill0, 1: fin0}

        units = [(b, h) for b in range(BPC) for h in range(H)]
        prev = None
        for i, u in enumerate(units):
            b, h = u
            if b == 1 and h == 0:
                # drain remaining b0 fillers (qk(b1) packs among them),
                # then emit the first two b1 head repacks
                for f in fills[0]:
                    f()
                fills[0] = []
                for hh in (0, 1):
                    qk_repack(1, 0, hh)
                    qk_repack(1, 1, hh)
            if b == 1 and h + 2 < H:
                # just-in-time repack: its head-pool WAR wait (energy of
                # head h) is nearly satisfied at emission time
                qk_repack(1, 0, h + 2)
                qk_repack(1, 1, h + 2)
            if b == 0:
                # force-drain fillers until this unit's heads are repacked
                # and (before unit h1, whose kt-loop emits attnv(h0)) the
                # batch-0 vhat half-1 sweep is complete
                while (0, "q", h) not in heads or (
                        h >= 1 and not state.get("vh1_0")):
                    fills[0].pop(0)()
            # kt-granular interleave: energy(u, kt) | attnv(prev, j=kt) |
            # filler, so the PE never sits behind a not-yet-computed exp
            # and the ScalarE pipeline stays fed
            fill = fills[b]
            budget = -(-len(fill) // max(1, H - h))
            emitted = 0
            for kt in range(NKT):
                energy_kt(u, kt)
                if prev is not None:
                    attnv_j(prev, kt)
                if emitted < budget and fill and kt % 2 == 1:
                    fill.pop(0)()
                    emitted += 1
            while emitted < budget and fill:
                fill.pop(0)()
                emitted += 1
            prev = u

        # last unit's attn@V with lag-1 interleave of final(b1): group jj
        # fires right after jj's transposes+copies, covering their latency
        for f in fills[1]:
            f()
        for jj in range(NKT):
            attnv_j(prev, jj)
            if jj >= 1:
                for half in range(2):
                    final_group(1, jj - 1, half)
        for half in range(2):
            final_group(1, NKT - 1, half)


def _get_runner(with_bias=False):
    """Build (once per variant) a jitted shard_map executing the NEFF."""
    key = ("runner", with_bias)
    if key in _CACHE:
        return _CACHE[key]

    import jax
    from jax.experimental.shard_map import shard_map
    from jax.sharding import Mesh, NamedSharding, PartitionSpec
    from concourse import mybir
    from concourse.bass2jax import (
        _bass_exec_p, install_neuronx_cc_hook, partition_id_tensor)

    nc = _build(with_bias=with_bias)
    install_neuronx_cc_hook()

    partition_name = (
        nc.partition_id_tensor.name if nc.partition_id_tensor else None)
    in_names, out_names, out_avals, zero_outs = [], [], [], []
    for alloc in nc.m.functions[0].allocations:
        if not isinstance(alloc, mybir.MemoryLocationSet):
            continue
        name = alloc.memorylocations[0].name
        if alloc.kind == "ExternalInput":
            if name != partition_name:
                in_names.append(name)
        elif alloc.kind == "ExternalOutput":
            out_names.append(name)
            shape = tuple(alloc.tensor_shape)
            dtype = mybir.dt.np(alloc.dtype)
            out_avals.append(jax.core.ShapedArray(shape, dtype))
            zero_outs.append(np.zeros(shape, dtype))
    n_params = len(in_names)
    all_in_names = in_names + out_names
    if partition_name is not None:
        all_in_names = all_in_names + [partition_name]

    def _bass_body(*args):
        operands = list(args)
        if partition_name is not None:
            operands.append(partition_id_tensor())
        outs = _bass_exec_p.bind(
            *operands,
            out_avals=tuple(out_avals),
            in_names=tuple(all_in_names),
            out_names=tuple(out_names),
            lowering_input_output_aliases=(),
            sim_require_finite=True,
            sim_require_nnan=True,
            nc=nc,
        )
        return tuple(outs)

    devices = jax.devices()[:NCORES]
    mesh = Mesh(np.asarray(devices), ("core",))
    spec = PartitionSpec("core")
    rspec = PartitionSpec()          # replicated (weights/biases)
    sharding = NamedSharding(mesh, spec)
    rsharding = NamedSharding(mesh, rspec)
    n_outs = len(out_names)
    # xT is per-core data; everything else is identical across cores
    in_specs = tuple(spec if nm == "xT" else rspec for nm in in_names)
    jitted = jax.jit(
        shard_map(
            _bass_body, mesh=mesh,
            in_specs=in_specs + (spec,) * n_outs,
            out_specs=(spec,) * n_outs,
            check_rep=False,
        ),
        keep_unused=True,
    )
    zeros_dev = [
        jax.device_put(np.concatenate([z] * NCORES, axis=0), sharding)
        for z in zero_outs
    ]
    runner = {
        "jitted": jitted, "in_names": in_names, "out_names": out_names,
        "sharding": sharding, "rsharding": rsharding,
        "zeros_dev": zeros_dev, "jax": jax,
    }
    _CACHE[key] = runner
    return runner


def _prep_inputs(x, Wq, bq, Wk, bk, Wv, bv, Wo, bo):
    """Host-side prep: arrays keyed by NEFF input name. xT is per-core
    concatenated; weights/biases are single copies (replicated spec)."""
    import ml_dtypes
    x = np.asarray(x, dtype=np.float32)
    Wq, Wk, Wv, Wo = (np.asarray(w, dtype=np.float32) for w in (Wq, Wk, Wv, Wo))
    bq, bk, bv, bo = (np.asarray(v, dtype=np.float32) for v in (bq, bk, bv, bo))

    xcat = np.ascontiguousarray(
        x.reshape(NCORES, T, E).transpose(0, 2, 1)).reshape(NCORES * E, T)
    # bqk [128, 2*KT]: column (i*KT + t) = rows 128t..128t+128 of bq/bk
    bqk = np.concatenate(
        [bq.reshape(KT, 128).T, bk.reshape(KT, 128).T], axis=1)

    return {
        "xT": xcat,
        "wq": Wq, "wk": Wk, "wv": Wv,
        "wob": Wo.astype(ml_dtypes.bfloat16),
        "bqk": np.ascontiguousarray(bqk),
        "bv1": np.ascontiguousarray(bv.reshape(1, E)),
        "bo1": bo.reshape(1, E).astype(ml_dtypes.bfloat16),
        "ident": np.eye(128, dtype=ml_dtypes.bfloat16),
    }


def _run(inputs, device_resident=None, with_bias=False):
    r = _get_runner(with_bias)
    args = []
    for name in r["in_names"]:
        if device_resident is not None and name in device_resident:
            args.append(device_resident[name])
        else:
            args.append(inputs[name])
    outs = r["jitted"](*args, *r["zeros_dev"])
    return {name: outs[i] for i, name in enumerate(r["out_names"])}


def _weights_on_device(inputs, with_bias=False):
    """device_put the (replicated) weight/bias arrays once per unique value."""
    import hashlib
    r = _get_runner(with_bias)
    key = hashlib.sha1()
    for name in sorted(inputs):
        if name == "xT":
            continue
        a = inputs[name]
        key.update(name.encode())
        key.update(a.shape.__repr__().encode())
        key.update(a.tobytes())
    key = key.hexdigest()
    cached = _CACHE.get("weights_dev")
    if cached is not None and cached[0] == key:
        return cached[1]
    dev = {
        name: r["jax"].device_put(a, r["rsharding"])
        for name, a in inputs.items() if name != "xT"
    }
    _CACHE["weights_dev"] = (key, dev)
    return dev


def kernel(x, Wq, bq, Wk, bk, Wv, bv, Wo, bo):
    with_bias = any(
        np.any(np.asarray(v)) for v in (bq, bk, bv, bo))
    inputs = _prep_inputs(x, Wq, bq, Wk, bk, Wv, bv, Wo, bo)
    dev = _weights_on_device(inputs, with_bias)
    outs = _run(inputs, dev, with_bias)
    out = np.asarray(outs["out"])          # [NCORES*T, E]
    return out.reshape(B, N, E)


def bench(x, Wq, bq, Wk, bk, Wv, bv, Wo, bo, iters=20):
    """Time repeated executions with all inputs device-resident."""
    import time
    r = _get_runner()
    inputs = _prep_inputs(x, Wq, bq, Wk, bk, Wv, bv, Wo, bo)
    dev = _weights_on_device(inputs)
    dev = dict(dev)
    dev["xT"] = r["jax"].device_put(inputs["xT"], r["sharding"])

    out = _run(inputs, dev)
    list(out.values())[0].block_until_ready()

    t0 = time.time()
    last = None
    for _ in range(iters):
        last = _run(inputs, dev)
    for v in last.values():
        v.block_until_ready()
    dt = (time.time() - t0) / iters
    return dt
